# revision 5
# baseline (speedup 1.0000x reference)
"""Trainium2 Bass kernel for nn_EnhancedLocalAttention.

Reference semantics (B=4, L=4096, C=1024, H=16, D=64, WIN=256, step=128):
  qkv = x @ W_qkv + b_qkv -> q,k,v [B,H,L,D]
  overlapping windows n: tokens [n*128, n*128+256)
  per (b,h,n): S = (Q_win^T K_win)/8  (D x D, contracted over the 256 window
  tokens), P = softmax(S, axis=-1), O = P @ V_win^T  (D x W)
  regroup: rows of reshape(O, [256, 64]) laid at tokens n*256..n*256+255,
  slice to L -> only windows 0..15 survive; then @ W_out + b_out.

Sharding: 8 cores = (4 batches) x (2 window-halves of 8 windows each).
Each core consumes 9 x 128-token chunks and produces 2048 output rows.

v4:
  - host pre-casts x/W to f16
  - x^T tiles via ONE DMA xbar transpose per chunk straight from DRAM
  - V^T computed directly with W-stationary matmuls (no V transposes)
  - P^T via identity-matmuls packed with tile_position
  - softmax row-sums on DVE (reduce_sum); copies split scalar/vector
  - Q and K projections as separate passes (half the PSUM residency),
    dedicated 4-bank GEMM pool + 4-bank window pool
  - window units ordered so ~3us of independent PE work sits between the
    S matmuls and the first P^T consumer (hides the softmax chain)
  - weights stream on two DMA queues (gpsimd + scalar)
"""

import threading

import numpy as np

import concourse.bacc as bacc
import concourse.masks as masks
import concourse.mybir as mybir
import concourse.tile as tile
from concourse._compat import get_trn_type
from concourse.bass_utils import run_bass_kernel_spmd

F32 = mybir.dt.float32
F16 = mybir.dt.float16
EXP = mybir.ActivationFunctionType.Exp
AXX = mybir.AxisListType.X

B, L, C = 4, 4096, 1024
H, D, WIN, STEP = 16, 64, 256, 128
NCHUNK = 9            # 128-token chunks per core
NWIN = 8              # windows per core
TOK = NCHUNK * 128    # 1152 input tokens per core
OUT_ROWS = NWIN * 256 # 2048 output rows per core


def interleave(a, b):
    """Merge two unit lists proportionally (Bresenham)."""
    if not b:
        return list(a)
    if not a:
        return list(b)
    out = []
    ia = ib = 0
    while ia < len(a) or ib < len(b):
        if ib >= len(b) or (ia < len(a) and ia * len(b) <= ib * len(a)):
            out.append(a[ia]); ia += 1
        else:
            out.append(b[ib]); ib += 1
    return out


def build_program(with_bias=False):
    nc = bacc.Bacc(
        get_trn_type() or "TRN2",
        target_bir_lowering=False,
        debug=False,
        num_devices=8,
    )
    xs = nc.dram_tensor("xs", [TOK, C], F16, kind="ExternalInput")
    wqkv = nc.dram_tensor("wqkv", [C, 3 * C], F16, kind="ExternalInput")
    bqkv = nc.dram_tensor("bqkv", [3 * C], F32, kind="ExternalInput")
    wout = nc.dram_tensor("wout", [C, C], F16, kind="ExternalInput")
    bout = nc.dram_tensor("bout", [C], F32, kind="ExternalInput")
    out = nc.dram_tensor("out", [OUT_ROWS, C], F32, kind="ExternalOutput")

    from contextlib import ExitStack

    with tile.TileContext(nc) as tc, ExitStack() as ctx:
        pool = lambda name, bufs: ctx.enter_context(tc.tile_pool(name=name, bufs=bufs))
        wq_pool = pool("wq", 8)
        wk_pool = pool("wk", 8)
        wv_pool = pool("wv", 8)
        wo_pool = pool("wo", 8)
        const_pool = pool("const", 1)
        xt_pool = pool("xt", 5)       # 5 chunks in flight, [128, 1024] each
        q_pool = pool("q", 5)
        k_pool = pool("k", 5)
        vt_pool = pool("vt", 5)
        at_pool = pool("at", 8)
        st_pool = pool("st", 8)
        yt_pool = pool("yt", 12)
        o_pool = pool("o", 3)
        ps_mm = ctx.enter_context(tc.tile_pool(name="psmm", bufs=4, space="PSUM"))
        ps_wn = ctx.enter_context(tc.tile_pool(name="pswn", bufs=4, space="PSUM"))

        # --- constants / weights ---
        idf16 = const_pool.tile([128, 128], F16, tag="idf16", name="idf16")
        masks.make_identity(nc, idf16[:])
        ones = const_pool.tile([1, 128], F16, tag="ones", name="ones")
        nc.vector.memset(ones[:], 1.0)
        bq_sb = const_pool.tile([1, 3 * C], F16, tag="bq", name="bq_sb")
        bo_sb = const_pool.tile([1, C], F16, tag="bo", name="bo_sb")
        if with_bias:
            nc.gpsimd.dma_start(bq_sb[:], bqkv.ap().rearrange("(a f) -> a f", a=1))
            nc.gpsimd.dma_start(bo_sb[:], bout.ap().rearrange("(a f) -> a f", a=1))

        # x^T per chunk as one [128, 8*128] tile; block cb = cols cb*128..+128
        xt_all = [None] * NCHUNK

        def prefetch_xt(r):
            xtt = xt_pool.tile([128, C], F16, tag="xt", name="xtt")
            nc.sync.dma_start(
                xtt[:].rearrange("p (b t) -> p b t", b=8),
                xs.ap()[r * 128 : (r + 1) * 128, :],
                transpose=True,
            )
            xt_all[r] = xtt

        prefetch_xt(0)
        prefetch_xt(1)
        prefetch_xt(2)

        # weights: Q columns on gpsimd queue, K columns on scalar queue,
        # then V (gpsimd) and W_out (scalar) -- two streams in parallel
        wq_sb, wk_sb = [], []
        for cb in range(8):
            tq = wq_pool.tile([128, C], F16, tag="wq", name=f"wq{cb}")
            nc.gpsimd.dma_start(tq[:], wqkv.ap()[cb * 128 : (cb + 1) * 128, 0:C])
            wq_sb.append(tq)
            tk = wk_pool.tile([128, C], F16, tag="wk", name=f"wk{cb}")
            nc.scalar.dma_start(tk[:], wqkv.ap()[cb * 128 : (cb + 1) * 128, C : 2 * C])
            wk_sb.append(tk)
        wv_sb, wo_sb = [], []
        for cb in range(8):
            tv = wv_pool.tile([128, C], F16, tag="wv", name=f"wv{cb}")
            nc.gpsimd.dma_start(
                tv[:], wqkv.ap()[cb * 128 : (cb + 1) * 128, 2 * C : 3 * C]
            )
            wv_sb.append(tv)
            to = wo_pool.tile([128, C], F16, tag="wo", name=f"wo{cb}")
            nc.scalar.dma_start(to[:], wout.ap()[cb * 128 : (cb + 1) * 128, :])
            wo_sb.append(to)

        q_sb = [None] * NCHUNK
        k_sb = [None] * NCHUNK
        vt_sb = [None] * NCHUNK  # [e-pair 128, hp*128 + tok]

        def qkv_units(r):
            """Emit-callback units for chunk r's Q, K, V^T projections."""
            st = {}

            def u_pref():
                if r + 3 < NCHUNK:
                    prefetch_xt(r + 3)
                st["xt"] = [
                    xt_all[r][:, cb * 128 : (cb + 1) * 128] for cb in range(8)
                ]

            def u_q_alloc():
                st["pq"] = [
                    ps_mm.tile([128, 512], F32, tag="mm", name=f"pq{i}")
                    for i in range(2)
                ]

            def u_q(cb):
                def f():
                    for i in range(2):
                        nc.tensor.matmul(
                            st["pq"][i][:],
                            st["xt"][cb],
                            wq_sb[cb][:, i * 512 : (i + 1) * 512],
                            start=(cb == 0),
                            stop=(not with_bias and cb == 7),
                        )
                return f

            def u_q_fin():
                if with_bias:
                    for i in range(2):
                        nc.tensor.matmul(
                            st["pq"][i][:],
                            ones[:, :],
                            bq_sb[:, i * 512 : (i + 1) * 512],
                            start=False,
                            stop=True,
                        )
                qt = q_pool.tile([128, C], F16, tag="q", name="qt")
                nc.scalar.mul(qt[:, 0:512], st["pq"][0][:], 0.125)
                nc.scalar.mul(qt[:, 512:1024], st["pq"][1][:], 0.125)
                q_sb[r] = qt

            def u_k_alloc():
                st["pk"] = [
                    ps_mm.tile([128, 512], F32, tag="mm", name=f"pk{i}")
                    for i in range(2)
                ]

            def u_k(cb):
                def f():
                    for i in range(2):
                        nc.tensor.matmul(
                            st["pk"][i][:],
                            st["xt"][cb],
                            wk_sb[cb][:, i * 512 : (i + 1) * 512],
                            start=(cb == 0),
                            stop=(not with_bias and cb == 7),
                        )
                return f

            def u_k_fin():
                if with_bias:
                    for i in range(2):
                        nc.tensor.matmul(
                            st["pk"][i][:],
                            ones[:, :],
                            bq_sb[:, C + i * 512 : C + (i + 1) * 512],
                            start=False,
                            stop=True,
                        )
                kt = k_pool.tile([128, C], F16, tag="k", name="kt")
                nc.vector.tensor_copy(kt[:, 0:512], st["pk"][0][:])
                nc.vector.tensor_copy(kt[:, 512:1024], st["pk"][1][:])
                k_sb[r] = kt

            def u_v_alloc():
                st["pv"] = [
                    ps_mm.tile([128, 512], F32, tag="mm", name=f"pv{i}")
                    for i in range(2)
                ]

            def u_v(hp):
                def f():
                    pv = st["pv"][hp // 4]
                    sl = (hp % 4) * 128
                    for cb in range(8):
                        nc.tensor.matmul(
                            pv[:, sl : sl + 128],
                            wv_sb[cb][:, hp * 128 : (hp + 1) * 128],
                            st["xt"][cb],
                            start=(cb == 0),
                            stop=(not with_bias and cb == 7),
                        )
                    if with_bias:
                        nc.tensor.matmul(
                            pv[:, sl : sl + 128],
                            bq_sb[:, 2 * C + hp * 128 : 2 * C + (hp + 1) * 128],
                            ones[:, :],
                            start=False,
                            stop=True,
                        )
                return f

            def u_v_fin():
                v_t = vt_pool.tile([128, C], F16, tag="vt", name="v_t")
                nc.vector.tensor_copy(v_t[:, 0:512], st["pv"][0][:])
                nc.scalar.copy(v_t[:, 512:1024], st["pv"][1][:])
                vt_sb[r] = v_t

            units = [u_pref, u_q_alloc]
            units += [u_q(cb) for cb in range(8)]
            units += [u_q_fin, u_k_alloc]
            units += [u_k(cb) for cb in range(8)]
            units += [u_k_fin, u_v_alloc]
            units += [u_v(hp) for hp in range(8)]
            units += [u_v_fin]
            return units

        def window_units(w):
            """Emit-callback units for window w (chunks w, w+1)."""
            yt = [None] * 8
            hps = [{} for _ in range(8)]
            prs = [{} for _ in range(4)]

            def u_s(hp):
                def f():
                    st = hps[hp]
                    pr = prs[hp // 2]
                    sw = ps_wn.tile([128, 512], F32, tag="wn", name="sw")
                    s = sw[:, 0:128]
                    for rr, (b0, b1) in ((w, (True, False)), (w + 1, (False, True))):
                        nc.tensor.matmul(
                            s,
                            q_sb[rr][:, hp * 128 : (hp + 1) * 128],
                            k_sb[rr][:, hp * 128 : (hp + 1) * 128],
                            start=b0,
                            stop=b1,
                        )
                    p_exp = at_pool.tile([128, 64], F16, tag="p_exp", name="p_exp")
                    nc.scalar.activation(p_exp[0:64, :], sw[0:64, 0:64], EXP)
                    nc.scalar.activation(p_exp[64:128, :], sw[64:128, 64:128], EXP)
                    if hp % 2 == 0:
                        pr["ssum2"] = st_pool.tile(
                            [128, 2], F32, tag="ssum", name="ssum2"
                        )
                    nc.vector.reduce_sum(
                        pr["ssum2"][:, hp % 2 : hp % 2 + 1], p_exp[:], axis=AXX
                    )
                    st["p_exp"] = p_exp
                return f

            def u_pt(pp):
                """Normalize + P^T for head-pairs 2pp, 2pp+1."""
                def f():
                    pr = prs[pp]
                    rs2 = st_pool.tile([128, 2], F32, tag="rs", name="rs2")
                    nc.vector.reciprocal(rs2[:], pr["ssum2"][:])
                    ptw = ps_wn.tile([128, 512], F32, tag="wn", name="ptw")
                    ptp2 = ptw[:, 0:128]
                    for i in (0, 1):
                        hp = 2 * pp + i
                        st = hps[hp]
                        p_n = at_pool.tile([128, 64], F16, tag="p_n", name="p_n")
                        nc.vector.tensor_scalar_mul(
                            p_n[:], st["p_exp"], rs2[:, i : i + 1]
                        )
                        nc.tensor.matmul(
                            ptp2[0:64, i * 64 : (i + 1) * 64],
                            p_n[0:64, :],
                            idf16[0:64, 0:64],
                            start=True,
                            stop=True,
                            tile_position=(0, 0),
                        )
                        nc.tensor.matmul(
                            ptp2[64:128, i * 64 : (i + 1) * 64],
                            p_n[64:128, :],
                            idf16[64:128, 64:128],
                            start=True,
                            stop=True,
                            tile_position=(64, 64),
                        )
                    ptsb2 = at_pool.tile([128, 128], F16, tag="ptsb", name="ptsb2")
                    nc.vector.tensor_copy(ptsb2[:], ptp2)
                    pr["ptsb2"] = ptsb2
                return f

            def u_o(hp):
                def f():
                    pr = prs[hp // 2]
                    yw = ps_wn.tile([128, 512], F32, tag="wn", name="yw")
                    ypsum = yw[:, 0:256]
                    c0 = (hp % 2) * 64
                    for po in (0, 64):
                        rh = pr["ptsb2"][po : po + 64, c0 : c0 + 64]
                        for wq in range(4):
                            vtt = vt_sb[w + wq // 2]
                            col = hp * 128 + (wq % 2) * 64
                            nc.tensor.matmul(
                                ypsum[po : po + 64, wq * 64 : (wq + 1) * 64],
                                vtt[po : po + 64, col : col + 64],
                                rh,
                                start=True,
                                stop=True,
                                tile_position=(po, po),
                            )
                    ytt = yt_pool.tile([128, 256], F16, tag="yt", name="ytt")
                    # Y^T[c, d*4+wq] = ypsum[c, wq*64+d]  (torch-unfold regroup)
                    eng = nc.vector.tensor_copy if hp % 2 else nc.scalar.copy
                    eng(
                        ytt[:].rearrange("p (b a) -> p a b", a=4),
                        ypsum.rearrange("p (a b) -> p a b", a=4),
                    )
                    yt[hp] = ytt
                return f

            def u_op(th):
                def f():
                    po_m = [
                        ps_mm.tile([128, 512], F32, tag="mm", name=f"pom{i}")
                        for i in range(2)
                    ]
                    for cb in range(8):
                        for mi in range(2):
                            nc.tensor.matmul(
                                po_m[mi][:],
                                yt[cb][:, th * 128 : (th + 1) * 128],
                                wo_sb[cb][:, mi * 512 : (mi + 1) * 512],
                                start=(cb == 0),
                                stop=(not with_bias and cb == 7),
                            )
                    if with_bias:
                        for mi in range(2):
                            nc.tensor.matmul(
                                po_m[mi][:],
                                ones[:, :],
                                bo_sb[:, mi * 512 : (mi + 1) * 512],
                                start=False,
                                stop=True,
                            )
                    ot = o_pool.tile([128, C], F32, tag="o", name="ot")
                    nc.vector.tensor_copy(ot[:, 0:512], po_m[0][:])
                    nc.scalar.copy(ot[:, 512:1024], po_m[1][:])
                    row = w * 256 + th * 128
                    nc.sync.dma_start(out.ap()[row : row + 128, :], ot[:])
                return f

            units = [u_s(0), u_s(1), u_s(2), u_s(3), u_s(4), u_pt(0)]
            units += [u_s(5), u_pt(1), u_o(0), u_s(6), u_pt(2), u_o(1)]
            units += [u_s(7), u_pt(3), u_o(2), u_o(3), u_o(4), u_o(5)]
            units += [u_o(6), u_o(7), u_op(0), u_op(1)]
            return units

        for r in range(NCHUNK + 1):
            qk = qkv_units(r) if r < NCHUNK else []
            win = window_units(r - 2) if 2 <= r < NWIN + 2 else []
            for u in interleave(qk, win):
                u()

    nc.compile()
    return nc


_CACHE = {}
_LOCK = threading.Lock()


def _get_program(with_bias=False):
    key = f"nc_bias{with_bias}"
    with _LOCK:
        if key not in _CACHE:
            _CACHE[key] = build_program(with_bias=with_bias)
        return _CACHE[key]


def make_in_maps(x, W_qkv, b_qkv, W_out, b_out):
    x16 = np.asarray(x, dtype=np.float16)
    wqkv16 = np.asarray(W_qkv, dtype=np.float16)
    wout16 = np.asarray(W_out, dtype=np.float16)
    bqkv = np.asarray(b_qkv, dtype=np.float32)
    bout = np.asarray(b_out, dtype=np.float32)
    in_maps = []
    for cid in range(8):
        b, half = cid // 2, cid % 2
        t0 = half * NWIN * STEP
        in_maps.append(
            {
                "xs": np.ascontiguousarray(x16[b, t0 : t0 + TOK, :]),
                "wqkv": wqkv16,
                "bqkv": bqkv,
                "wout": wout16,
                "bout": bout,
            }
        )
    return in_maps


def kernel(x, W_qkv, b_qkv, W_out, b_out):
    with_bias = bool(np.any(b_qkv)) or bool(np.any(b_out))
    nc = _get_program(with_bias=with_bias)
    in_maps = make_in_maps(x, W_qkv, b_qkv, W_out, b_out)
    res = run_bass_kernel_spmd(nc, in_maps, core_ids=list(range(8)))
    out_full = np.empty((B, L, C), dtype=np.float32)
    for cid in range(8):
        b, half = cid // 2, cid % 2
        out_full[b, half * OUT_ROWS : (half + 1) * OUT_ROWS, :] = res.results[cid][
            "out"
        ]
    return out_full
